# revision 4
# baseline (speedup 1.0000x reference)
"""5-layer GAT on 8 TRN2 NeuronCores — dst-sharded gather/aggregate kernel.

Strategy (per core c of 8):
  - dst nodes [c*6250, (c+1)*6250) and their (dst-sorted) edges, grouped in
    128-dst windows; window edges split by src half (int16 gather idx limit),
    padded to 128-edge chunks (uniform structure across cores for SPMD).
  - Scatter/expand masks (S: edge->dst, ST: dst->edge) are layer-invariant
    pure structure: precomputed on host as bf16, streamed from DRAM per call
    (keeps the DVE off the critical path entirely).
  - Per layer: dma_gather [z|el] rows (bf16, el fp32-packed) from the
    replicated table; er expanded via PE matmul (lhsT=ST chunk);
    e = leaky_relu(el+er) via scalar-engine Prelu, ex = Exp -> bf16;
    aggregate sum(ex*z) and sum(ex) in one PE matmul per chunk into PSUM.
  - Window flush: divide by sum (DVE tensor_scalar with per-partition
    reciprocal), +bias, elu = relu(x) + min(exp(x),1) - 1 (scalar Exp/Relu),
    transpose h on PE, then immediately the next layer's z matmul for this
    window -> cc_in; AllGather the packed [z|el] table after the last window.
  - L1 z-phase is sharded too (own 6250 nodes) + AllGather, not replicated.
Numerics: z/el in bf16/fp32-packed as in the validated baseline; ex bf16.
"""
import numpy as np

N = 50000
E = 800000
NC = 8
SH = N // NC
WIN = 128
NWIN = (SH + WIN - 1) // WIN
HALF = 25000
LAYERS = [(128, 64, 4), (256, 64, 2), (128, 64, 2), (128, 64, 1), (64, 4, 1)]
NEG = 0.2
MAXCH = 8


def _row_units(hf, h):
    u = hf + 2 * h
    return ((u + 127) // 128) * 128


ROWS_U = [_row_units(h * f, h) for (_, f, h) in LAYERS]


def _prep(src, dst):
    order = np.argsort(dst, kind="stable")
    src_s, dst_s = src[order], dst[order]
    core_of = dst_s // SH
    core_lists = []
    nch = np.zeros((NWIN, 2), np.int64)
    for c in range(NC):
        m = core_of == c
        s, d = src_s[m], dst_s[m] - c * SH
        w = d // WIN
        lists = {}
        for wi in range(NWIN):
            mw = w == wi
            sw, dw = s[mw], d[mw]
            lo = sw < HALF
            lists[(wi, 0)] = (sw[lo].astype(np.int64), dw[lo])
            lists[(wi, 1)] = (sw[~lo].astype(np.int64) - HALF, dw[~lo])
            for hf in range(2):
                nch[wi, hf] = max(nch[wi, hf],
                                  (len(lists[(wi, hf)][0]) + 127) // 128)
        core_lists.append(lists)
    nch = np.maximum(nch, 1)

    calls = []            # (win, half, c0, g)
    win_first = {}
    win_last = {}
    c0 = 0
    for wi in range(NWIN):
        win_first[wi] = c0
        for hf in range(2):
            n = int(nch[wi, hf])
            k = 0
            while k < n:
                g = min(MAXCH, n - k)
                calls.append((wi, hf, c0 + k, g))
                k += g
            c0 += n
        win_last[wi] = c0 - 1
    NCH = c0
    EPAD = NCH * 128

    idx_streams, smasks, stmasks = [], [], []
    dcol = np.arange(128, dtype=np.int64)
    for c in range(NC):
        lists = core_lists[c]
        idx = np.zeros(EPAD, np.int64)
        ld = np.full(EPAD, 999, np.int64)
        pos = 0
        for wi in range(NWIN):
            for hf in range(2):
                s, d = lists[(wi, hf)]
                n = int(nch[wi, hf]) * 128
                idx[pos:pos + len(s)] = s
                ld[pos:pos + len(d)] = d % WIN
                pos += n
        blocks = []
        for (wi, hf, cc0, g) in calls:
            blk = idx[cc0 * 128:(cc0 + g) * 128].astype(np.int16)
            blocks.append(np.tile(blk.reshape(-1, 16).T, (8, 1)))
        idx_streams.append(np.ascontiguousarray(np.concatenate(blocks, axis=1)))

        import ml_dtypes
        bf = ml_dtypes.bfloat16
        ld2 = ld.reshape(NCH, 128)
        # S[p, kg*128+d] = (ld[kg*128+p] == d)
        sm = (ld2[:, :, None] == dcol[None, None, :])
        sm = np.ascontiguousarray(
            sm.transpose(1, 0, 2).reshape(128, NCH * 128).astype(bf))
        # ST[p, kg*128+e] = (ld[kg*128+e] == p)
        st = (dcol[:, None, None] == ld2[None, :, :])
        st = np.ascontiguousarray(st.reshape(128, NCH * 128).astype(bf))
        smasks.append(sm)
        stmasks.append(st)

    return calls, win_first, win_last, NCH, idx_streams, smasks, stmasks


def _build(calls, win_first, win_last, NCH):
    from contextlib import ExitStack
    import concourse.bass as bass
    import concourse.bacc as bacc
    import concourse.tile as tile
    from concourse import mybir
    from concourse.masks import make_identity

    F32, BF16, I16 = mybir.dt.float32, mybir.dt.bfloat16, mybir.dt.int16
    A = mybir.ActivationFunctionType
    MAX = mybir.AluOpType.max
    MULT = mybir.AluOpType.mult
    ADD = mybir.AluOpType.add
    MIN = mybir.AluOpType.min
    TOT16 = NCH * 8

    nc = bacc.Bacc("TRN2", num_devices=NC, num_swdge_queues=4)

    xTs = nc.dram_tensor("xTs", [128, SH], BF16, kind="ExternalInput")
    Waug, btens = [], []
    for li, (fin, fo, h) in enumerate(LAYERS, 1):
        hf = h * fo
        Waug.append(nc.dram_tensor(f"Waug{li}", [fin, hf + 2 * h], BF16,
                                   kind="ExternalInput"))
        btens.append(nc.dram_tensor(f"bb{li}", [1, hf], F32, kind="ExternalInput"))
    idxs_d = nc.dram_tensor("idxs", [128, TOT16], I16, kind="ExternalInput")
    sm_d = nc.dram_tensor("sm", [128, NCH * 128], BF16, kind="ExternalInput")
    stm_d = nc.dram_tensor("stm", [128, NCH * 128], BF16, kind="ExternalInput")
    out_d = nc.dram_tensor("out", [SH, 4], F32, kind="ExternalOutput")

    T, cc_in = [], []
    for li in range(1, 6):
        u = ROWS_U[li - 1]
        cc_in.append(nc.dram_tensor(f"ccin{li}", [SH, u], BF16, kind="Internal"))
        T.append(nc.dram_tensor(f"T{li}", [N, u], BF16, kind="Internal",
                                addr_space="Shared"))
    rg = [list(range(NC))]

    with tile.TileContext(nc) as tc:
        with ExitStack() as ctx:
            cpool = ctx.enter_context(tc.tile_pool(name="const", bufs=1))
            gpool = ctx.enter_context(tc.tile_pool(name="gat", bufs=6))
            spool = ctx.enter_context(tc.tile_pool(name="masks", bufs=8))
            rpool = ctx.enter_context(tc.tile_pool(name="rhs", bufs=4))
            epool = ctx.enter_context(tc.tile_pool(name="expx", bufs=8))
            wpool = ctx.enter_context(tc.tile_pool(name="wflush", bufs=6))
            zpool = ctx.enter_context(tc.tile_pool(name="zphase", bufs=4))
            pp_w = ctx.enter_context(tc.tile_pool(name="ps_w", bufs=2, space="PSUM"))
            pp_er = ctx.enter_context(tc.tile_pool(name="ps_er", bufs=2, space="PSUM"))
            pp_z = ctx.enter_context(tc.tile_pool(name="ps_z", bufs=2, space="PSUM"))
            pp_t = ctx.enter_context(tc.tile_pool(name="ps_t", bufs=2, space="PSUM"))

            ident = cpool.tile([128, 128], BF16)
            make_identity(nc, ident[:, :])

            idx_sb = cpool.tile([128, TOT16], I16)
            nc.sync.dma_start(out=idx_sb[:, :], in_=idxs_d[:, :])

            wsb, bsb = [], []
            for li, (fin, fo, h) in enumerate(LAYERS, 1):
                hf = h * fo
                cols = hf + 2 * h
                kch = (fin + 127) // 128
                wt = cpool.tile([128, kch, cols], BF16, tag=f"w{li}")
                if kch > 1:
                    nc.sync.dma_start(
                        out=wt[:, :, :],
                        in_=Waug[li - 1][:, :].rearrange("(k p) c -> p k c", p=128))
                else:
                    nc.sync.dma_start(out=wt[:fin, 0, :], in_=Waug[li - 1][:, :])
                wsb.append(wt)
                bt = cpool.tile([128, hf], F32, tag=f"b{li}")
                bsrc = btens[li - 1][:, :]
                nc.sync.dma_start(out=bt[:, :], in_=bass.AP(
                    tensor=bsrc.tensor, offset=bsrc.offset,
                    ap=[[0, 128]] + [list(p) for p in bsrc.ap[1:]]))
                bsb.append(bt)

            er_sh = cpool.tile([128, NWIN, 4], BF16)
            nc.vector.memset(er_sh[:, :, :], 0.0)

            xts_sb = cpool.tile([128, SH], BF16)
            nc.sync.dma_start(out=xts_sb[:, :], in_=xTs[:, :])

            # ---------- L1 z phase: own shard -> cc_in -> AllGather T1 ----
            fin, fo, h = LAYERS[0]
            hf = h * fo
            ru = ROWS_U[0]
            for wi in range(NWIN):
                m = min(WIN, SH - wi * WIN)
                ps = pp_z.tile([128, hf + 2 * h], F32, tag="psz")
                nc.tensor.matmul(ps[:m, :], lhsT=xts_sb[:, wi * WIN:wi * WIN + m],
                                 rhs=wsb[0][:, 0, :], start=True, stop=True)
                row_t = zpool.tile([128, ru], BF16, tag="rowt")
                nc.scalar.activation(row_t[:m, :hf], ps[:m, :hf], A.Copy)
                nc.scalar.activation(row_t[:m, hf:hf + 2 * h].bitcast(F32),
                                     ps[:m, hf:hf + h], A.Copy)
                nc.scalar.activation(er_sh[:m, wi, :h], ps[:m, hf + h:hf + 2 * h],
                                     A.Copy)
                nc.sync.dma_start(out=cc_in[0][wi * WIN:wi * WIN + m, :],
                                  in_=row_t[:m, :])
            nc.gpsimd.collective_compute(
                "AllGather", mybir.AluOpType.bypass, rg,
                ins=[cc_in[0][:, :]], outs=[T[0][:, :]])
            tc.strict_bb_all_engine_barrier()

            # ---------- layers ----------
            for li, (fin, fo, h) in enumerate(LAYERS, 1):
                hf = h * fo
                ru = ROWS_U[li - 1]
                tbl = T[li - 1]
                psw = None
                off16 = 0
                for ci, (wi, half, c0, g) in enumerate(calls):
                    ni = g * 128
                    if c0 == win_first[wi]:
                        psw = pp_w.tile([128, hf + h], F32, tag="psw")
                    base = tbl[0:HALF, :] if half == 0 else tbl[HALF:N, :]
                    g_t = gpool.tile([128, MAXCH, ru], BF16, tag="gt")
                    nc.gpsimd.dma_gather(
                        g_t[:, :g, :], base, idx_sb[:, off16:off16 + g * 8],
                        num_idxs=ni, num_idxs_reg=ni, elem_size=ru,
                        queue_num=ci % 4)
                    off16 += g * 8

                    S_t = spool.tile([128, MAXCH * 128], BF16, tag="S")
                    nc.sync.dma_start(out=S_t[:, :ni],
                                      in_=sm_d[:, c0 * 128:c0 * 128 + ni])
                    ST_t = spool.tile([128, MAXCH * 128], BF16, tag="ST")
                    nc.sync.dma_start(out=ST_t[:, :ni],
                                      in_=stm_d[:, c0 * 128:c0 * 128 + ni])

                    er_ps = pp_er.tile([128, MAXCH * 4], F32, tag="erps")
                    for k in range(g):
                        nc.tensor.matmul(er_ps[:, k * h:(k + 1) * h],
                                         lhsT=ST_t[:, k * 128:(k + 1) * 128],
                                         rhs=er_sh[:, wi, :h], start=True, stop=True)

                    e_t = epool.tile([128, MAXCH * 4], F32, tag="e")
                    elv = g_t[:, :g, hf:hf + 2 * h].bitcast(F32)
                    ev = bass.AP(tensor=e_t[:, :].tensor, offset=e_t[:, :].offset,
                                 ap=[[MAXCH * 4, 128], [h, g], [1, h]])
                    erv = bass.AP(tensor=er_ps[:, :].tensor,
                                  offset=er_ps[:, :].offset,
                                  ap=[[MAXCH * 4, 128], [h, g], [1, h]])
                    nc.vector.tensor_tensor(out=ev, in0=elv, in1=erv, op=ADD)
                    lk = epool.tile([128, MAXCH * 4], F32, tag="lk")
                    nc.scalar.activation(lk[:, :g * h], e_t[:, :g * h], A.Prelu,
                                         alpha=NEG)
                    ex = epool.tile([128, MAXCH * 4], BF16, tag="ex")
                    nc.scalar.activation(ex[:, :g * h], lk[:, :g * h], A.Exp)

                    rhs_t = rpool.tile([128, MAXCH, hf + h], BF16, tag="rhs")
                    for hi in range(h):
                        exv = bass.AP(tensor=ex[:, :].tensor,
                                      offset=ex[:, :].offset + hi,
                                      ap=[[MAXCH * 4, 128], [h, g], [0, fo]])
                        nc.vector.tensor_tensor(
                            out=rhs_t[:, :g, hi * fo:(hi + 1) * fo],
                            in0=g_t[:, :g, hi * fo:(hi + 1) * fo], in1=exv, op=MULT)
                    exv2 = bass.AP(tensor=ex[:, :].tensor, offset=ex[:, :].offset,
                                   ap=[[MAXCH * 4, 128], [h, g], [1, h]])
                    nc.vector.tensor_copy(rhs_t[:, :g, hf:hf + h], exv2)

                    for k in range(g):
                        kg = c0 + k
                        nc.tensor.matmul(psw[:, :],
                                         lhsT=S_t[:, k * 128:(k + 1) * 128],
                                         rhs=rhs_t[:, k, :],
                                         start=(kg == win_first[wi]),
                                         stop=(kg == win_last[wi]))

                    if c0 + g - 1 == win_last[wi]:
                        # -------- window flush --------
                        m = min(WIN, SH - wi * WIN)
                        sg = wpool.tile([128, 4], F32, tag="sg")
                        nc.vector.tensor_scalar(out=sg[:m, :h],
                                                in0=psw[:m, hf:hf + h],
                                                scalar1=1e-30, scalar2=None,
                                                op0=MAX)
                        rr = wpool.tile([128, 4], F32, tag="rr")
                        nc.vector.reciprocal(rr[:m, :h], sg[:m, :h])
                        ow = wpool.tile([128, hf], F32, tag="ow")
                        for hi in range(h):
                            nc.vector.tensor_scalar(
                                out=ow[:m, hi * fo:(hi + 1) * fo],
                                in0=psw[:m, hi * fo:(hi + 1) * fo],
                                scalar1=rr[:m, hi:hi + 1], scalar2=None,
                                op0=MULT)
                        nc.vector.tensor_add(ow[:m, :], ow[:m, :], bsb[li - 1][:m, :])
                        if li == 5:
                            nc.sync.dma_start(out=out_d[wi * WIN:wi * WIN + m, :],
                                              in_=ow[:m, :4])
                        else:
                            texp = wpool.tile([128, hf], F32, tag="texp")
                            nc.scalar.activation(texp[:m, :], ow[:m, :], A.Exp)
                            nc.vector.tensor_scalar(out=texp[:m, :], in0=texp[:m, :],
                                                    scalar1=1.0, scalar2=-1.0,
                                                    op0=MIN, op1=ADD)
                            trl = wpool.tile([128, hf], F32, tag="trl")
                            nc.scalar.activation(trl[:m, :], ow[:m, :], A.Relu)
                            hbf = wpool.tile([128, hf], BF16, tag="hbf")
                            nc.vector.tensor_tensor(out=hbf[:m, :], in0=texp[:m, :],
                                                    in1=trl[:m, :], op=ADD)
                            # transpose h for next layer's z matmul
                            kch2 = (hf + 127) // 128
                            hTw = wpool.tile([128, 2, 128], BF16, tag="hTw")
                            for k in range(kch2):
                                kk = min(128, hf - k * 128)
                                pt = pp_t.tile([128, 128], BF16, tag="pt")
                                nc.tensor.transpose(
                                    pt[:kk, :m], hbf[:m, k * 128:k * 128 + kk],
                                    ident[:m, :m])
                                nc.scalar.activation(hTw[:kk, k, :m], pt[:kk, :m],
                                                     A.Copy)
                            # next layer z for this window
                            fin2, fo2, h2 = LAYERS[li]
                            hf2 = h2 * fo2
                            ru2 = ROWS_U[li]
                            ps2 = pp_z.tile([128, hf2 + 2 * h2], F32, tag="psz")
                            for k in range(kch2):
                                kk = min(128, hf - k * 128)
                                nc.tensor.matmul(ps2[:m, :], lhsT=hTw[:kk, k, :m],
                                                 rhs=wsb[li][:kk, k, :],
                                                 start=(k == 0), stop=(k == kch2 - 1))
                            row_t = zpool.tile([128, ru2], BF16, tag="rowt2")
                            nc.scalar.activation(row_t[:m, :hf2], ps2[:m, :hf2],
                                                 A.Copy)
                            nc.scalar.activation(
                                row_t[:m, hf2:hf2 + 2 * h2].bitcast(F32),
                                ps2[:m, hf2:hf2 + h2], A.Copy)
                            nc.scalar.activation(er_sh[:m, wi, :h2],
                                                 ps2[:m, hf2 + h2:hf2 + 2 * h2],
                                                 A.Copy)
                            nc.sync.dma_start(out=cc_in[li][wi * WIN:wi * WIN + m, :],
                                              in_=row_t[:m, :])

                if li < 5:
                    nc.gpsimd.collective_compute(
                        "AllGather", mybir.AluOpType.bypass, rg,
                        ins=[cc_in[li][:, :]], outs=[T[li][:, :]])
                    tc.strict_bb_all_engine_barrier()
    nc.finalize()
    return nc


_CACHE = {}
_LAST_RES = None


def kernel(**inputs):
    import ml_dtypes

    x = np.asarray(inputs["x"], np.float32)
    src = np.asarray(inputs["src"], np.int64)
    dst = np.asarray(inputs["dst"], np.int64)

    calls, win_first, win_last, NCH, idx_streams, smasks, stmasks = _prep(src, dst)

    key = (NCH, len(calls))
    if key not in _CACHE:
        _CACHE[key] = _build(calls, win_first, win_last, NCH)
    nc = _CACHE[key]

    bf = ml_dtypes.bfloat16
    common = {}
    for li, (fin, fo, h) in enumerate(LAYERS, 1):
        W = np.asarray(inputs[f"W{li}"], np.float32)
        al = np.asarray(inputs[f"al{li}"], np.float32)
        ar = np.asarray(inputs[f"ar{li}"], np.float32)
        b = np.asarray(inputs[f"b{li}"], np.float32)
        Wr = W.reshape(fin, h, fo)
        wl = np.einsum("ihf,hf->ih", Wr, al)
        wr = np.einsum("ihf,hf->ih", Wr, ar)
        common[f"Waug{li}"] = np.ascontiguousarray(
            np.concatenate([W, wl, wr], axis=1)).astype(bf)
        common[f"bb{li}"] = np.ascontiguousarray(b.reshape(1, -1))

    in_maps = []
    for c in range(NC):
        m = dict(common)
        m["xTs"] = np.ascontiguousarray(x[c * SH:(c + 1) * SH].T).astype(bf)
        m["idxs"] = idx_streams[c]
        m["sm"] = smasks[c]
        m["stm"] = stmasks[c]
        in_maps.append(m)

    from concourse.bass_utils import run_bass_kernel_spmd
    res = run_bass_kernel_spmd(nc, in_maps, core_ids=list(range(NC)))
    global _LAST_RES
    _LAST_RES = res
    out = np.concatenate([res.results[c]["out"] for c in range(NC)], axis=0)
    return out.astype(np.float32)


if __name__ == "__main__":
    data = np.load("/tmp/inputs.npz")
    out = kernel(**{k: data[k] for k in data.files})
    exp = np.load("/tmp/expected.npy")
    rel = np.abs(out - exp) / np.abs(exp).max()
    print("rel err:", rel.max(), "mean", rel.mean())


# revision 9
# speedup vs baseline: 1.3201x; 1.3201x over previous
"""5-layer GAT on 8 TRN2 NeuronCores — dst-sharded gather/aggregate kernel.

Strategy (per core c of 8):
  - dst nodes [c*6250, (c+1)*6250) and their (dst-sorted) edges, grouped in
    128-dst windows; window edges split by src half (int16 gather idx limit),
    padded to 128-edge chunks (uniform structure across cores for SPMD).
  - Scatter/expand masks (S: edge->dst, ST: dst->edge) are layer-invariant
    pure structure: precomputed on host as bf16, streamed from DRAM per call
    (keeps the DVE off the critical path entirely).
  - Per layer: dma_gather [z|el] rows (bf16, el fp32-packed) from the
    replicated table; er expanded via PE matmul (lhsT=ST chunk);
    e = leaky_relu(el+er) via scalar-engine Prelu, ex = Exp -> bf16;
    aggregate sum(ex*z) and sum(ex) in one PE matmul per chunk into PSUM.
  - Window flush: divide by sum (DVE tensor_scalar with per-partition
    reciprocal), +bias, elu = relu(x) + min(exp(x),1) - 1 (scalar Exp/Relu),
    transpose h on PE, then immediately the next layer's z matmul for this
    window -> cc_in; AllGather the packed [z|el] table after the last window.
  - L1 z-phase is sharded too (own 6250 nodes) + AllGather, not replicated.
Numerics: z/el in bf16/fp32-packed as in the validated baseline; ex bf16.
"""
import numpy as np

N = 50000
E = 800000
NC = 8
SH = N // NC
WIN = 128
NWIN = (SH + WIN - 1) // WIN
HALF = 25000
LAYERS = [(128, 64, 4), (256, 64, 2), (128, 64, 2), (128, 64, 1), (64, 4, 1)]
NEG = 0.2
MAXCH = 8


def _row_units(hf, h):
    u = hf + 2 * h
    return ((u + 127) // 128) * 128


ROWS_U = [_row_units(h * f, h) for (_, f, h) in LAYERS]


def _prep(src, dst):
    order = np.argsort(dst, kind="stable")
    src_s, dst_s = src[order], dst[order]
    core_of = dst_s // SH
    core_lists = []
    nch = np.zeros((NWIN, 2), np.int64)
    for c in range(NC):
        m = core_of == c
        s, d = src_s[m], dst_s[m] - c * SH
        w = d // WIN
        lists = {}
        for wi in range(NWIN):
            mw = w == wi
            sw, dw = s[mw], d[mw]
            lo = sw < HALF
            lists[(wi, 0)] = (sw[lo].astype(np.int64), dw[lo])
            lists[(wi, 1)] = (sw[~lo].astype(np.int64) - HALF, dw[~lo])
            for hf in range(2):
                nch[wi, hf] = max(nch[wi, hf],
                                  (len(lists[(wi, hf)][0]) + 127) // 128)
        core_lists.append(lists)
    nch = np.maximum(nch, 1)

    calls = []            # (win, half, c0, g)
    win_first = {}
    win_last = {}
    c0 = 0
    for wi in range(NWIN):
        win_first[wi] = c0
        for hf in range(2):
            n = int(nch[wi, hf])
            k = 0
            while k < n:
                g = min(MAXCH, n - k)
                calls.append((wi, hf, c0 + k, g))
                k += g
            c0 += n
        win_last[wi] = c0 - 1
    NCH = c0
    EPAD = NCH * 128

    idx_streams, smasks, stmasks = [], [], []
    dcol = np.arange(128, dtype=np.int64)
    for c in range(NC):
        lists = core_lists[c]
        idx = np.zeros(EPAD, np.int64)
        ld = np.full(EPAD, 999, np.int64)
        pos = 0
        for wi in range(NWIN):
            for hf in range(2):
                s, d = lists[(wi, hf)]
                n = int(nch[wi, hf]) * 128
                idx[pos:pos + len(s)] = s
                ld[pos:pos + len(d)] = d % WIN
                pos += n
        blocks = []
        for (wi, hf, cc0, g) in calls:
            blk = idx[cc0 * 128:(cc0 + g) * 128].astype(np.int16)
            blocks.append(np.tile(blk.reshape(-1, 16).T, (8, 1)))
        idx_streams.append(np.ascontiguousarray(np.concatenate(blocks, axis=1)))

        import ml_dtypes
        bf = ml_dtypes.float8_e4m3
        ld2 = ld.reshape(NCH, 128)
        # S[p, kg*128+d] = (ld[kg*128+p] == d)
        sm = (ld2[:, :, None] == dcol[None, None, :])
        sm = np.ascontiguousarray(
            sm.transpose(1, 0, 2).reshape(128, NCH * 128).astype(bf))
        # ST[p, kg*128+e] = (ld[kg*128+e] == p)
        st = (dcol[:, None, None] == ld2[None, :, :])
        st = np.ascontiguousarray(st.reshape(128, NCH * 128).astype(bf))
        smasks.append(sm)
        stmasks.append(st)

    return calls, win_first, win_last, NCH, idx_streams, smasks, stmasks


def _build(calls, win_first, win_last, NCH):
    from contextlib import ExitStack
    import concourse.bass as bass
    import concourse.bacc as bacc
    import concourse.tile as tile
    from concourse import mybir
    from concourse.masks import make_identity

    F32, BF16, I16 = mybir.dt.float32, mybir.dt.bfloat16, mybir.dt.int16
    FP8 = mybir.dt.float8e4
    A = mybir.ActivationFunctionType
    MAX = mybir.AluOpType.max
    MULT = mybir.AluOpType.mult
    ADD = mybir.AluOpType.add
    MIN = mybir.AluOpType.min
    TOT16 = NCH * 8

    nc = bacc.Bacc("TRN2", num_devices=NC, num_swdge_queues=4)

    xTs = nc.dram_tensor("xTs", [128, SH], BF16, kind="ExternalInput")
    Waug, btens = [], []
    for li, (fin, fo, h) in enumerate(LAYERS, 1):
        hf = h * fo
        Waug.append(nc.dram_tensor(f"Waug{li}", [fin, hf + 2 * h], BF16,
                                   kind="ExternalInput"))
        btens.append(nc.dram_tensor(f"bb{li}", [1, hf], F32, kind="ExternalInput"))
    idxs_d = nc.dram_tensor("idxs", [128, TOT16], I16, kind="ExternalInput")
    sm_d = nc.dram_tensor("sm", [128, NCH * 128], FP8, kind="ExternalInput")
    stm_d = nc.dram_tensor("stm", [128, NCH * 128], FP8, kind="ExternalInput")
    out_d = nc.dram_tensor("out", [SH, 4], F32, kind="ExternalOutput")

    T, cc_in = [], []
    for li in range(1, 6):
        u = ROWS_U[li - 1]
        cc_in.append(nc.dram_tensor(f"ccin{li}", [SH, u], BF16, kind="Internal"))
        T.append(nc.dram_tensor(f"T{li}", [N, u], BF16, kind="Internal",
                                addr_space="Shared"))
    rg = [list(range(NC))]

    with tile.TileContext(nc) as tc:
        with ExitStack() as ctx:
            cpool = ctx.enter_context(tc.tile_pool(name="const", bufs=1))
            gpool = ctx.enter_context(tc.tile_pool(name="gat", bufs=8))
            spool = ctx.enter_context(tc.tile_pool(name="masks", bufs=8))
            rpool = ctx.enter_context(tc.tile_pool(name="rhs", bufs=4))
            epool = ctx.enter_context(tc.tile_pool(name="expx", bufs=8))
            wpool = ctx.enter_context(tc.tile_pool(name="wflush", bufs=6))
            zpool = ctx.enter_context(tc.tile_pool(name="zphase", bufs=4))
            pp_w = ctx.enter_context(tc.tile_pool(name="ps_w", bufs=2, space="PSUM"))
            pp_er = ctx.enter_context(tc.tile_pool(name="ps_er", bufs=2, space="PSUM"))
            pp_z = ctx.enter_context(tc.tile_pool(name="ps_z", bufs=1, space="PSUM"))
            pp_t = ctx.enter_context(tc.tile_pool(name="ps_t", bufs=1, space="PSUM"))
            pp_x = ctx.enter_context(tc.tile_pool(name="ps_x", bufs=2, space="PSUM"))

            ident = cpool.tile([128, 128], BF16)
            make_identity(nc, ident[:, :])

            idx_sb = cpool.tile([128, TOT16], I16)
            nc.sync.dma_start(out=idx_sb[:, :], in_=idxs_d[:, :])

            wsb, bsb = [], []
            for li, (fin, fo, h) in enumerate(LAYERS, 1):
                hf = h * fo
                cols = hf + 2 * h
                kch = (fin + 127) // 128
                wt = cpool.tile([128, kch, cols], BF16, tag=f"w{li}")
                if kch > 1:
                    nc.sync.dma_start(
                        out=wt[:, :, :],
                        in_=Waug[li - 1][:, :].rearrange("(k p) c -> p k c", p=128))
                else:
                    nc.sync.dma_start(out=wt[:fin, 0, :], in_=Waug[li - 1][:, :])
                wsb.append(wt)
                bt = cpool.tile([128, hf], F32, tag=f"b{li}")
                bsrc = btens[li - 1][:, :]
                nc.sync.dma_start(out=bt[:, :], in_=bass.AP(
                    tensor=bsrc.tensor, offset=bsrc.offset,
                    ap=[[0, 128]] + [list(p) for p in bsrc.ap[1:]]))
                bsb.append(bt)

            er_sh = cpool.tile([128, NWIN, 4], BF16)
            nc.vector.memset(er_sh[:, :, :], 0.0)

            xts_sb = cpool.tile([128, SH], BF16)
            nc.sync.dma_start(out=xts_sb[:, :], in_=xTs[:, :])

            # ---------- L1 z phase: own shard -> cc_in -> AllGather T1 ----
            fin, fo, h = LAYERS[0]
            hf = h * fo
            ru = ROWS_U[0]
            for wi in range(NWIN):
                m = min(WIN, SH - wi * WIN)
                ps = pp_z.tile([128, hf + 2 * h], F32, tag="psz")
                nc.tensor.matmul(ps[:m, :], lhsT=xts_sb[:, wi * WIN:wi * WIN + m],
                                 rhs=wsb[0][:, 0, :], start=True, stop=True)
                row_t = zpool.tile([128, ru], BF16, tag="rowt")
                nc.scalar.activation(row_t[:m, :hf], ps[:m, :hf], A.Copy)
                nc.scalar.activation(row_t[:m, hf:hf + 2 * h].bitcast(F32),
                                     ps[:m, hf:hf + h], A.Copy)
                nc.scalar.activation(er_sh[:m, wi, :h], ps[:m, hf + h:hf + 2 * h],
                                     A.Copy)
                nc.sync.dma_start(out=cc_in[0][wi * WIN:wi * WIN + m, :],
                                  in_=row_t[:m, :])
            nc.gpsimd.collective_compute(
                "AllGather", mybir.AluOpType.bypass, rg,
                ins=[cc_in[0][:, :]], outs=[T[0][:, :]])
            tc.strict_bb_all_engine_barrier()

            # ---------- layers ----------
            for li, (fin, fo, h) in enumerate(LAYERS, 1):
                hf = h * fo
                ru = ROWS_U[li - 1]
                tbl = T[li - 1]
                psw = None
                off16 = 0
                for ci, (wi, half, c0, g) in enumerate(calls):
                    ni = g * 128
                    if c0 == win_first[wi]:
                        psw = pp_w.tile([128, hf], F32, tag="psw")
                        psx = pp_x.tile([128, 4], F32, tag="psx")
                    base = tbl[0:HALF, :] if half == 0 else tbl[HALF:N, :]
                    g_t = gpool.tile([128, MAXCH, ru], BF16, tag="gt")
                    nc.gpsimd.dma_gather(
                        g_t[:, :g, :], base, idx_sb[:, off16:off16 + g * 8],
                        num_idxs=ni, num_idxs_reg=ni, elem_size=ru,
                        queue_num=ci % 4)
                    off16 += g * 8

                    S_t = spool.tile([128, MAXCH * 128], FP8, tag="S")
                    nc.sync.dma_start(out=S_t[:, :ni],
                                      in_=sm_d[:, c0 * 128:c0 * 128 + ni])
                    ST_t = spool.tile([128, MAXCH * 128], FP8, tag="ST")
                    nc.scalar.dma_start(out=ST_t[:, :ni],
                                        in_=stm_d[:, c0 * 128:c0 * 128 + ni])

                    er_ps = pp_er.tile([128, MAXCH * 4], F32, tag="erps")
                    for k in range(g):
                        nc.tensor.matmul(er_ps[:, k * h:(k + 1) * h],
                                         lhsT=ST_t[:, k * 128:(k + 1) * 128],
                                         rhs=er_sh[:, wi, :h], start=True, stop=True)

                    e_t = epool.tile([128, MAXCH * 4], F32, tag="e")
                    elv = g_t[:, :g, hf:hf + 2 * h].bitcast(F32)
                    ev = bass.AP(tensor=e_t[:, :].tensor, offset=e_t[:, :].offset,
                                 ap=[[MAXCH * 4, 128], [h, g], [1, h]])
                    erv = bass.AP(tensor=er_ps[:, :].tensor,
                                  offset=er_ps[:, :].offset,
                                  ap=[[MAXCH * 4, 128], [h, g], [1, h]])
                    nc.vector.tensor_tensor(out=ev, in0=elv, in1=erv, op=ADD)
                    lk = epool.tile([128, MAXCH * 4], F32, tag="lk")
                    nc.scalar.activation(lk[:, :g * h], e_t[:, :g * h], A.Prelu,
                                         alpha=NEG)
                    ex = epool.tile([128, MAXCH * 4], BF16, tag="ex")
                    nc.scalar.activation(ex[:, :g * h], lk[:, :g * h], A.Exp)

                    rhs_t = rpool.tile([128, MAXCH, hf], BF16, tag="rhs")
                    for hi in range(h):
                        exv = bass.AP(tensor=ex[:, :].tensor,
                                      offset=ex[:, :].offset + hi,
                                      ap=[[MAXCH * 4, 128], [h, g], [0, fo]])
                        nc.vector.tensor_tensor(
                            out=rhs_t[:, :g, hi * fo:(hi + 1) * fo],
                            in0=g_t[:, :g, hi * fo:(hi + 1) * fo], in1=exv, op=MULT)

                    for k in range(g):
                        kg = c0 + k
                        st_flags = dict(start=(kg == win_first[wi]),
                                        stop=(kg == win_last[wi]))
                        nc.tensor.matmul(psw[:, :hf],
                                         lhsT=S_t[:, k * 128:(k + 1) * 128],
                                         rhs=rhs_t[:, k, :], **st_flags)
                        nc.tensor.matmul(psx[:, :h],
                                         lhsT=S_t[:, k * 128:(k + 1) * 128],
                                         rhs=ex[:, k * h:(k + 1) * h], **st_flags)

                    if c0 + g - 1 == win_last[wi]:
                        # -------- window flush --------
                        m = min(WIN, SH - wi * WIN)
                        sg = wpool.tile([128, 4], F32, tag="sg")
                        nc.vector.tensor_scalar(out=sg[:m, :h],
                                                in0=psx[:m, :h],
                                                scalar1=1e-30, scalar2=None,
                                                op0=MAX)
                        rr = wpool.tile([128, 4], F32, tag="rr")
                        nc.vector.reciprocal(rr[:m, :h], sg[:m, :h])
                        ow = wpool.tile([128, hf], F32, tag="ow")
                        for hi in range(h):
                            nc.vector.tensor_scalar(
                                out=ow[:m, hi * fo:(hi + 1) * fo],
                                in0=psw[:m, hi * fo:(hi + 1) * fo],
                                scalar1=rr[:m, hi:hi + 1], scalar2=None,
                                op0=MULT)
                        nc.vector.tensor_add(ow[:m, :], ow[:m, :], bsb[li - 1][:m, :])
                        if li == 5:
                            nc.sync.dma_start(out=out_d[wi * WIN:wi * WIN + m, :],
                                              in_=ow[:m, :4])
                        else:
                            texp = wpool.tile([128, hf], F32, tag="texp")
                            nc.scalar.activation(texp[:m, :], ow[:m, :], A.Exp)
                            nc.vector.tensor_scalar(out=texp[:m, :], in0=texp[:m, :],
                                                    scalar1=1.0, scalar2=-1.0,
                                                    op0=MIN, op1=ADD)
                            trl = wpool.tile([128, hf], F32, tag="trl")
                            nc.scalar.activation(trl[:m, :], ow[:m, :], A.Relu)
                            hbf = wpool.tile([128, hf], BF16, tag="hbf")
                            nc.vector.tensor_tensor(out=hbf[:m, :], in0=texp[:m, :],
                                                    in1=trl[:m, :], op=ADD)
                            # transpose h for next layer's z matmul
                            kch2 = (hf + 127) // 128
                            hTw = wpool.tile([128, 2, 128], BF16, tag="hTw")
                            for k in range(kch2):
                                kk = min(128, hf - k * 128)
                                pt = pp_t.tile([128, 128], BF16, tag="pt")
                                nc.tensor.transpose(
                                    pt[:kk, :m], hbf[:m, k * 128:k * 128 + kk],
                                    ident[:m, :m])
                                nc.scalar.activation(hTw[:kk, k, :m], pt[:kk, :m],
                                                     A.Copy)
                            # next layer z for this window
                            fin2, fo2, h2 = LAYERS[li]
                            hf2 = h2 * fo2
                            ru2 = ROWS_U[li]
                            ps2 = pp_z.tile([128, hf2 + 2 * h2], F32, tag="psz")
                            for k in range(kch2):
                                kk = min(128, hf - k * 128)
                                nc.tensor.matmul(ps2[:m, :], lhsT=hTw[:kk, k, :m],
                                                 rhs=wsb[li][:kk, k, :],
                                                 start=(k == 0), stop=(k == kch2 - 1))
                            row_t = zpool.tile([128, ru2], BF16, tag="rowt2")
                            nc.scalar.activation(row_t[:m, :hf2], ps2[:m, :hf2],
                                                 A.Copy)
                            nc.scalar.activation(
                                row_t[:m, hf2:hf2 + 2 * h2].bitcast(F32),
                                ps2[:m, hf2:hf2 + h2], A.Copy)
                            nc.scalar.activation(er_sh[:m, wi, :h2],
                                                 ps2[:m, hf2 + h2:hf2 + 2 * h2],
                                                 A.Copy)
                            nc.sync.dma_start(out=cc_in[li][wi * WIN:wi * WIN + m, :],
                                              in_=row_t[:m, :])

                if li < 5:
                    nc.gpsimd.collective_compute(
                        "AllGather", mybir.AluOpType.bypass, rg,
                        ins=[cc_in[li][:, :]], outs=[T[li][:, :]])
                    tc.strict_bb_all_engine_barrier()
    nc.finalize()
    return nc


_CACHE = {}
_LAST_RES = None


def kernel(**inputs):
    import ml_dtypes

    x = np.asarray(inputs["x"], np.float32)
    src = np.asarray(inputs["src"], np.int64)
    dst = np.asarray(inputs["dst"], np.int64)

    calls, win_first, win_last, NCH, idx_streams, smasks, stmasks = _prep(src, dst)

    key = (NCH, len(calls))
    if key not in _CACHE:
        _CACHE[key] = _build(calls, win_first, win_last, NCH)
    nc = _CACHE[key]

    bf = ml_dtypes.bfloat16
    common = {}
    for li, (fin, fo, h) in enumerate(LAYERS, 1):
        W = np.asarray(inputs[f"W{li}"], np.float32)
        al = np.asarray(inputs[f"al{li}"], np.float32)
        ar = np.asarray(inputs[f"ar{li}"], np.float32)
        b = np.asarray(inputs[f"b{li}"], np.float32)
        Wr = W.reshape(fin, h, fo)
        wl = np.einsum("ihf,hf->ih", Wr, al)
        wr = np.einsum("ihf,hf->ih", Wr, ar)
        common[f"Waug{li}"] = np.ascontiguousarray(
            np.concatenate([W, wl, wr], axis=1)).astype(bf)
        common[f"bb{li}"] = np.ascontiguousarray(b.reshape(1, -1))

    in_maps = []
    for c in range(NC):
        m = dict(common)
        m["xTs"] = np.ascontiguousarray(x[c * SH:(c + 1) * SH].T).astype(bf)
        m["idxs"] = idx_streams[c]
        m["sm"] = smasks[c]
        m["stm"] = stmasks[c]
        in_maps.append(m)

    from concourse.bass_utils import run_bass_kernel_spmd
    res = run_bass_kernel_spmd(nc, in_maps, core_ids=list(range(NC)))
    global _LAST_RES
    _LAST_RES = res
    out = np.concatenate([res.results[c]["out"] for c in range(NC)], axis=0)
    return out.astype(np.float32)


if __name__ == "__main__":
    data = np.load("/tmp/inputs.npz")
    out = kernel(**{k: data[k] for k in data.files})
    exp = np.load("/tmp/expected.npy")
    rel = np.abs(out - exp) / np.abs(exp).max()
    print("rel err:", rel.max(), "mean", rel.mean())


# revision 15
# speedup vs baseline: 1.5749x; 1.1930x over previous
"""5-layer GAT on 8 TRN2 NeuronCores — dst-sharded gather/aggregate kernel.

Strategy (per core c of 8):
  - dst nodes [c*6250, (c+1)*6250) and their (dst-sorted) edges, grouped in
    128-dst windows; window edges split by src half (int16 gather idx limit),
    padded to 128-edge chunks (uniform structure across cores for SPMD).
  - Scatter/expand masks (S: edge->dst, ST: dst->edge) are layer-invariant
    pure structure: precomputed on host as bf16, streamed from DRAM per call
    (keeps the DVE off the critical path entirely).
  - Per layer: dma_gather [z|el] rows (bf16, el fp32-packed) from the
    replicated table; er expanded via PE matmul (lhsT=ST chunk);
    e = leaky_relu(el+er) via scalar-engine Prelu, ex = Exp -> bf16;
    aggregate sum(ex*z) and sum(ex) in one PE matmul per chunk into PSUM.
  - Window flush: divide by sum (DVE tensor_scalar with per-partition
    reciprocal), +bias, elu = relu(x) + min(exp(x),1) - 1 (scalar Exp/Relu),
    transpose h on PE, then immediately the next layer's z matmul for this
    window -> cc_in; AllGather the packed [z|el] table after the last window.
  - L1 z-phase is sharded too (own 6250 nodes) + AllGather, not replicated.
Numerics: z/el in bf16/fp32-packed as in the validated baseline; ex bf16.
"""
import numpy as np

N = 50000
E = 800000
NC = 8
SH = N // NC
WIN = 128
NWIN = (SH + WIN - 1) // WIN
HALF = 25000
LAYERS = [(128, 64, 4), (256, 64, 2), (128, 64, 2), (128, 64, 1), (64, 4, 1)]
NEG = 0.2
MAXCH = 8


def _row_units(hf, h):
    u = hf + 2 * h
    return ((u + 127) // 128) * 128


ROWS_U = [_row_units(h * f, h) for (_, f, h) in LAYERS]


def _prep(src, dst):
    order = np.argsort(dst, kind="stable")
    src_s, dst_s = src[order], dst[order]
    core_of = dst_s // SH
    core_lists = []
    nch = np.zeros((NWIN, 2), np.int64)
    for c in range(NC):
        m = core_of == c
        s, d = src_s[m], dst_s[m] - c * SH
        w = d // WIN
        lists = {}
        for wi in range(NWIN):
            mw = w == wi
            sw, dw = s[mw], d[mw]
            lo = sw < HALF
            lists[(wi, 0)] = (sw[lo].astype(np.int64), dw[lo])
            lists[(wi, 1)] = (sw[~lo].astype(np.int64) - HALF, dw[~lo])
            for hf in range(2):
                nch[wi, hf] = max(nch[wi, hf],
                                  (len(lists[(wi, hf)][0]) + 127) // 128)
        core_lists.append(lists)
    nch = np.maximum(nch, 1)

    calls = []            # (win, half, c0, g, nreal) nreal = max real idx count
    win_first = {}
    win_last = {}
    c0 = 0
    for wi in range(NWIN):
        win_first[wi] = c0
        for hf in range(2):
            n = int(nch[wi, hf])
            nreal_grp = max(max(len(cl[(wi, hf)][0]) for cl in core_lists), 1)
            k = 0
            while k < n:
                g = min(MAXCH, n - k)
                nr = min(max(nreal_grp - k * 128, 1), g * 128)
                nr = min(((nr + 127) // 128) * 128, g * 128)
                calls.append((wi, hf, c0 + k, g, nr))
                k += g
            c0 += n
        win_last[wi] = c0 - 1
    NCH = c0
    EPAD = NCH * 128

    idx_streams, smasks, stmasks = [], [], []
    dcol = np.arange(128, dtype=np.int64)
    for c in range(NC):
        lists = core_lists[c]
        idx = np.zeros(EPAD, np.int64)
        ld = np.full(EPAD, 999, np.int64)
        pos = 0
        for wi in range(NWIN):
            for hf in range(2):
                s, d = lists[(wi, hf)]
                n = int(nch[wi, hf]) * 128
                idx[pos:pos + len(s)] = s
                ld[pos:pos + len(d)] = d % WIN
                pos += n
        blocks = []
        for (wi, hf, cc0, g, nr) in calls:
            blk = idx[cc0 * 128:(cc0 + g) * 128].astype(np.int16)
            blocks.append(np.tile(blk.reshape(-1, 16).T, (8, 1)))
        idx_streams.append(np.ascontiguousarray(np.concatenate(blocks, axis=1)))

        import ml_dtypes
        bf = ml_dtypes.float8_e4m3
        ld2 = ld.reshape(NCH, 128)
        # S[p, kg*128+d] = (ld[kg*128+p] == d)
        sm = (ld2[:, :, None] == dcol[None, None, :])
        sm = np.ascontiguousarray(
            sm.transpose(1, 0, 2).reshape(128, NCH * 128).astype(bf))
        # ST[p, kg*128+e] = (ld[kg*128+e] == p)
        st = (dcol[:, None, None] == ld2[None, :, :])
        st = np.ascontiguousarray(st.reshape(128, NCH * 128).astype(bf))
        smasks.append(sm)
        stmasks.append(st)

    return calls, win_first, win_last, NCH, idx_streams, smasks, stmasks


def _build(calls, win_first, win_last, NCH):
    from contextlib import ExitStack
    import concourse.bass as bass
    import concourse.bacc as bacc
    import concourse.tile as tile
    from concourse import mybir
    from concourse.masks import make_identity

    F32, BF16, I16 = mybir.dt.float32, mybir.dt.bfloat16, mybir.dt.int16
    FP8 = mybir.dt.float8e4
    A = mybir.ActivationFunctionType
    MAX = mybir.AluOpType.max
    MULT = mybir.AluOpType.mult
    ADD = mybir.AluOpType.add
    MIN = mybir.AluOpType.min
    TOT16 = NCH * 8

    nc = bacc.Bacc("TRN2", num_devices=NC, num_swdge_queues=4)

    xTs = nc.dram_tensor("xTs", [128, SH], BF16, kind="ExternalInput")
    Waug, btens = [], []
    for li, (fin, fo, h) in enumerate(LAYERS, 1):
        hf = h * fo
        Waug.append(nc.dram_tensor(f"Waug{li}", [fin, hf + 2 * h], BF16,
                                   kind="ExternalInput"))
        btens.append(nc.dram_tensor(f"bb{li}", [1, hf], F32, kind="ExternalInput"))
    idxs_d = nc.dram_tensor("idxs", [128, TOT16], I16, kind="ExternalInput")
    sm_d = nc.dram_tensor("sm", [128, NCH * 128], FP8, kind="ExternalInput")
    stm_d = nc.dram_tensor("stm", [128, NCH * 128], FP8, kind="ExternalInput")
    out_d = nc.dram_tensor("out", [SH, 4], F32, kind="ExternalOutput")

    T, cc_in = [], []
    for li in range(1, 6):
        u = ROWS_U[li - 1]
        cc_in.append(nc.dram_tensor(f"ccin{li}", [SH, u], BF16, kind="Internal"))
        T.append(nc.dram_tensor(f"T{li}", [N, u], BF16, kind="Internal",
                                addr_space="Shared"))
    rg = [list(range(NC))]

    with tile.TileContext(nc) as tc:
        with ExitStack() as ctx:
            cpool = ctx.enter_context(tc.tile_pool(name="const", bufs=1))
            gpool = ctx.enter_context(tc.tile_pool(name="gat", bufs=10))
            spool = ctx.enter_context(tc.tile_pool(name="masks", bufs=10))
            rpool = ctx.enter_context(tc.tile_pool(name="rhs", bufs=6))
            epool = ctx.enter_context(tc.tile_pool(name="expx", bufs=10))
            wpool = ctx.enter_context(tc.tile_pool(name="wflush", bufs=8))
            zpool = ctx.enter_context(tc.tile_pool(name="zphase", bufs=4))
            pp_w = ctx.enter_context(tc.tile_pool(name="ps_w", bufs=2, space="PSUM"))
            pp_er = ctx.enter_context(tc.tile_pool(name="ps_er", bufs=2, space="PSUM"))
            pp_z = ctx.enter_context(tc.tile_pool(name="ps_z", bufs=1, space="PSUM"))
            pp_t = ctx.enter_context(tc.tile_pool(name="ps_t", bufs=1, space="PSUM"))
            pp_x = ctx.enter_context(tc.tile_pool(name="ps_x", bufs=2, space="PSUM"))

            ident = cpool.tile([128, 128], BF16)
            make_identity(nc, ident[:, :])

            idx_sb = cpool.tile([128, TOT16], I16)
            nc.sync.dma_start(out=idx_sb[:, :], in_=idxs_d[:, :])

            wsb, bsb = [], []
            for li, (fin, fo, h) in enumerate(LAYERS, 1):
                hf = h * fo
                cols = hf + 2 * h
                kch = (fin + 127) // 128
                wt = cpool.tile([128, kch, cols], BF16, tag=f"w{li}")
                if kch > 1:
                    nc.sync.dma_start(
                        out=wt[:, :, :],
                        in_=Waug[li - 1][:, :].rearrange("(k p) c -> p k c", p=128))
                else:
                    nc.sync.dma_start(out=wt[:fin, 0, :], in_=Waug[li - 1][:, :])
                wsb.append(wt)
                bt = cpool.tile([128, hf], F32, tag=f"b{li}")
                bsrc = btens[li - 1][:, :]
                nc.sync.dma_start(out=bt[:, :], in_=bass.AP(
                    tensor=bsrc.tensor, offset=bsrc.offset,
                    ap=[[0, 128]] + [list(p) for p in bsrc.ap[1:]]))
                bsb.append(bt)

            er_sh = cpool.tile([128, NWIN, 4], BF16)
            nc.vector.memset(er_sh[:, :, :], 0.0)



            xts_sb = cpool.tile([128, SH], BF16)
            nc.sync.dma_start(out=xts_sb[:, :], in_=xTs[:, :])

            # ---------- L1 z phase: own shard -> cc_in -> AllGather T1 ----
            fin, fo, h = LAYERS[0]
            hf = h * fo
            ru = ROWS_U[0]
            for wi in range(NWIN):
                m = min(WIN, SH - wi * WIN)
                ps = pp_z.tile([128, hf + 2 * h], F32, tag="psz")
                nc.tensor.matmul(ps[:m, :], lhsT=xts_sb[:, wi * WIN:wi * WIN + m],
                                 rhs=wsb[0][:, 0, :], start=True, stop=True)
                row_t = zpool.tile([128, ru], BF16, tag="rowt")
                nc.scalar.activation(row_t[:m, :hf], ps[:m, :hf], A.Copy)
                nc.scalar.activation(row_t[:m, hf:hf + 2 * h].bitcast(F32),
                                     ps[:m, hf:hf + h], A.Copy)
                nc.scalar.activation(er_sh[:m, wi, :h], ps[:m, hf + h:hf + 2 * h],
                                     A.Copy)
                nc.sync.dma_start(out=cc_in[0][wi * WIN:wi * WIN + m, :],
                                  in_=row_t[:m, :])
            nc.gpsimd.collective_compute(
                "AllGather", mybir.AluOpType.bypass, rg,
                ins=[cc_in[0][:, :]], outs=[T[0][:, :]])
            tc.strict_bb_all_engine_barrier()

            # ---------- layers ----------
            for li, (fin, fo, h) in enumerate(LAYERS, 1):
                hf = h * fo
                ru = ROWS_U[li - 1]
                tbl = T[li - 1]
                psw = None
                off16 = 0
                for ci, (wi, half, c0, g, nreal) in enumerate(calls):
                    ni = g * 128 if li == 1 else nreal
                    nch_used = (ni + 127) // 128
                    if c0 == win_first[wi]:
                        psw = pp_w.tile([128, hf], F32, tag="psw")
                        psx = pp_x.tile([128, 4], F32, tag="psx")
                    base = tbl[0:HALF, :] if half == 0 else tbl[HALF:N, :]
                    g_t = gpool.tile([128, MAXCH, ru], BF16, tag="gt")
                    nc.gpsimd.dma_gather(
                        g_t[:, :nch_used, :], base, idx_sb[:, off16:off16 + g * 8],
                        num_idxs=ni, num_idxs_reg=ni, elem_size=ru,
                        queue_num=ci % 4)
                    off16 += g * 8

                    S_t = spool.tile([128, MAXCH * 128], FP8, tag="S")
                    nc.sync.dma_start(out=S_t[:, :ni],
                                      in_=sm_d[:, c0 * 128:c0 * 128 + ni])
                    ST_t = spool.tile([128, MAXCH * 128], FP8, tag="ST")
                    nc.scalar.dma_start(out=ST_t[:, :ni],
                                        in_=stm_d[:, c0 * 128:c0 * 128 + ni])

                    gk = nch_used
                    er_ps = pp_er.tile([128, MAXCH * 4], F32, tag="erps")
                    for k in range(gk):
                        nc.tensor.matmul(er_ps[:, k * h:(k + 1) * h],
                                         lhsT=ST_t[:, k * 128:(k + 1) * 128],
                                         rhs=er_sh[:, wi, :h], start=True, stop=True)

                    e_t = epool.tile([128, MAXCH * 4], F32, tag="e")
                    elv = g_t[:, :gk, hf:hf + 2 * h].bitcast(F32)
                    ev = bass.AP(tensor=e_t[:, :].tensor, offset=e_t[:, :].offset,
                                 ap=[[MAXCH * 4, 128], [h, gk], [1, h]])
                    erv = bass.AP(tensor=er_ps[:, :].tensor,
                                  offset=er_ps[:, :].offset,
                                  ap=[[MAXCH * 4, 128], [h, gk], [1, h]])
                    nc.vector.tensor_tensor(out=ev, in0=elv, in1=erv, op=ADD)
                    lk = epool.tile([128, MAXCH * 4], F32, tag="lk")
                    nc.scalar.activation(lk[:, :gk * h], e_t[:, :gk * h], A.Prelu,
                                         alpha=NEG)
                    ex = epool.tile([128, MAXCH * 4], BF16, tag="ex")
                    nc.scalar.activation(ex[:, :gk * h], lk[:, :gk * h], A.Exp)

                    rhs_t = rpool.tile([128, MAXCH, hf], BF16, tag="rhs")
                    exv = bass.AP(tensor=ex[:, :].tensor, offset=ex[:, :].offset,
                                  ap=[[MAXCH * 4, 128], [h, gk], [1, h], [0, fo]])
                    gv = bass.AP(tensor=g_t[:, :, :].tensor,
                                 offset=g_t[:, :, :].offset,
                                 ap=[[MAXCH * ru, 128], [ru, gk], [fo, h], [1, fo]])
                    rv = bass.AP(tensor=rhs_t[:, :, :].tensor,
                                 offset=rhs_t[:, :, :].offset,
                                 ap=[[MAXCH * hf, 128], [hf, gk], [fo, h], [1, fo]])
                    nc.vector.tensor_tensor(out=rv, in0=gv, in1=exv, op=MULT)

                    last_call = (c0 + g - 1 == win_last[wi])
                    for k in range(gk):
                        kg = c0 + k
                        st_flags = dict(start=(kg == win_first[wi]),
                                        stop=(last_call and k == gk - 1))
                        nc.tensor.matmul(psw[:, :hf],
                                         lhsT=S_t[:, k * 128:(k + 1) * 128],
                                         rhs=rhs_t[:, k, :], **st_flags)
                        nc.tensor.matmul(psx[:, :h],
                                         lhsT=S_t[:, k * 128:(k + 1) * 128],
                                         rhs=ex[:, k * h:(k + 1) * h], **st_flags)

                    if c0 + g - 1 == win_last[wi]:
                        # -------- window flush --------
                        m = min(WIN, SH - wi * WIN)
                        sg = wpool.tile([128, 4], F32, tag="sg")
                        nc.vector.tensor_scalar(out=sg[:m, :h],
                                                in0=psx[:m, :h],
                                                scalar1=1e-30, scalar2=None,
                                                op0=MAX)
                        rr = wpool.tile([128, 4], F32, tag="rr")
                        nc.vector.reciprocal(rr[:m, :h], sg[:m, :h])
                        ow = wpool.tile([128, hf], F32, tag="ow")
                        for hi in range(h):
                            nc.scalar.activation(
                                ow[:m, hi * fo:(hi + 1) * fo],
                                psw[:m, hi * fo:(hi + 1) * fo],
                                A.Identity, scale=rr[:m, hi:hi + 1])
                        nc.vector.tensor_add(ow[:m, :], ow[:m, :], bsb[li - 1][:m, :])
                        if li == 5:
                            nc.sync.dma_start(out=out_d[wi * WIN:wi * WIN + m, :],
                                              in_=ow[:m, :4])
                        else:
                            texp = wpool.tile([128, hf], F32, tag="texp")
                            nc.scalar.activation(texp[:m, :], ow[:m, :], A.Exp)
                            nc.vector.tensor_scalar(out=texp[:m, :], in0=texp[:m, :],
                                                    scalar1=1.0, scalar2=-1.0,
                                                    op0=MIN, op1=ADD)
                            trl = wpool.tile([128, hf], F32, tag="trl")
                            nc.scalar.activation(trl[:m, :], ow[:m, :], A.Relu)
                            hbf = wpool.tile([128, hf], BF16, tag="hbf")
                            nc.vector.tensor_tensor(out=hbf[:m, :], in0=texp[:m, :],
                                                    in1=trl[:m, :], op=ADD)
                            # transpose h for next layer's z matmul
                            kch2 = (hf + 127) // 128
                            hTw = wpool.tile([128, 2, 128], BF16, tag="hTw")
                            for k in range(kch2):
                                kk = min(128, hf - k * 128)
                                pt = pp_t.tile([128, 128], BF16, tag="pt")
                                nc.tensor.transpose(
                                    pt[:kk, :m], hbf[:m, k * 128:k * 128 + kk],
                                    ident[:m, :m])
                                nc.scalar.activation(hTw[:kk, k, :m], pt[:kk, :m],
                                                     A.Copy)
                            # next layer z for this window
                            fin2, fo2, h2 = LAYERS[li]
                            hf2 = h2 * fo2
                            ru2 = ROWS_U[li]
                            ps2 = pp_z.tile([128, hf2 + 2 * h2], F32, tag="psz")
                            for k in range(kch2):
                                kk = min(128, hf - k * 128)
                                nc.tensor.matmul(ps2[:m, :], lhsT=hTw[:kk, k, :m],
                                                 rhs=wsb[li][:kk, k, :],
                                                 start=(k == 0), stop=(k == kch2 - 1))
                            row_t = zpool.tile([128, ru2], BF16, tag="rowt2")
                            nc.scalar.activation(row_t[:m, :hf2], ps2[:m, :hf2],
                                                 A.Copy)
                            nc.scalar.activation(
                                row_t[:m, hf2:hf2 + 2 * h2].bitcast(F32),
                                ps2[:m, hf2:hf2 + h2], A.Copy)
                            nc.scalar.activation(er_sh[:m, wi, :h2],
                                                 ps2[:m, hf2 + h2:hf2 + 2 * h2],
                                                 A.Copy)
                            nc.sync.dma_start(out=cc_in[li][wi * WIN:wi * WIN + m, :],
                                              in_=row_t[:m, :])

                if li < 5:
                    nc.gpsimd.collective_compute(
                        "AllGather", mybir.AluOpType.bypass, rg,
                        ins=[cc_in[li][:, :]], outs=[T[li][:, :]])
                    tc.strict_bb_all_engine_barrier()
    nc.finalize()
    return nc


_CACHE = {}
_LAST_RES = None


def kernel(**inputs):
    import ml_dtypes

    x = np.asarray(inputs["x"], np.float32)
    src = np.asarray(inputs["src"], np.int64)
    dst = np.asarray(inputs["dst"], np.int64)

    calls, win_first, win_last, NCH, idx_streams, smasks, stmasks = _prep(src, dst)

    key = (NCH, len(calls))
    if key not in _CACHE:
        _CACHE[key] = _build(calls, win_first, win_last, NCH)
    nc = _CACHE[key]

    bf = ml_dtypes.bfloat16
    common = {}
    for li, (fin, fo, h) in enumerate(LAYERS, 1):
        W = np.asarray(inputs[f"W{li}"], np.float32)
        al = np.asarray(inputs[f"al{li}"], np.float32)
        ar = np.asarray(inputs[f"ar{li}"], np.float32)
        b = np.asarray(inputs[f"b{li}"], np.float32)
        Wr = W.reshape(fin, h, fo)
        wl = np.einsum("ihf,hf->ih", Wr, al)
        wr = np.einsum("ihf,hf->ih", Wr, ar)
        common[f"Waug{li}"] = np.ascontiguousarray(
            np.concatenate([W, wl, wr], axis=1)).astype(bf)
        common[f"bb{li}"] = np.ascontiguousarray(b.reshape(1, -1))

    in_maps = []
    for c in range(NC):
        m = dict(common)
        m["xTs"] = np.ascontiguousarray(x[c * SH:(c + 1) * SH].T).astype(bf)
        m["idxs"] = idx_streams[c]
        m["sm"] = smasks[c]
        m["stm"] = stmasks[c]
        in_maps.append(m)

    from concourse.bass_utils import run_bass_kernel_spmd
    res = run_bass_kernel_spmd(nc, in_maps, core_ids=list(range(NC)))
    global _LAST_RES
    _LAST_RES = res
    out = np.concatenate([res.results[c]["out"] for c in range(NC)], axis=0)
    return out.astype(np.float32)


if __name__ == "__main__":
    data = np.load("/tmp/inputs.npz")
    out = kernel(**{k: data[k] for k in data.files})
    exp = np.load("/tmp/expected.npy")
    rel = np.abs(out - exp) / np.abs(exp).max()
    print("rel err:", rel.max(), "mean", rel.mean())
